# revision 12
# baseline (speedup 1.0000x reference)
"""BlurDegradation kernel for 8x TRN2 NeuronCores.

Math: t[b] successive 11x11 depthwise *circular* convolutions compose into a
single circular convolution whose spectrum is the product of the per-step
spectra. The host composes the (tiny) 20 step-kernels into 21 cumulative
spectra with numpy FFTs and selects per-sample spectrum FK[t[b]]; each device
then computes, per image,  out = Re( F* . (FK o (F x F)) . F* ) / N^2  as four
chained dense matmul stages on the PE array (plus DVE/ACT/GpSimd pointwise
work). Contractions always run over the partition dim, so each stage
implicitly transposes and no PE/DMA transposes are needed.

Row-count optimizations vs the naive 4-stage dense chain:
 - Hermitian half-spectrum: y-frequencies k=0..257 only (258 = even, required
   by fp32r); fold weights (1,2,..,2,1,0) and 1/N^2 live in FK (host-side).
 - Stage 2 is matrix-stationary so the k-half axis is the cheap *moving* dim
   (N=258).
 - Stage 3 uses Gauss's 3-mult complex multiply (n1/n2/n3 chains on
   Ztr / Zq=Zti-Ztr / Zp=Ztr+Zti).
 - Stage 4 is matrix-stationary: natural [y,x] output, k-half contraction
   as K tiles (128, 2, 128).
 - H and the n1 PSUM tiles are staged to SBUF by the Scalar engine; the
   pointwise complex multiply runs as 8 wide [128, 4*258] ops split across
   Vector and GpSimd.

Sharding: pure data parallel, 8 samples per core, no cross-core comms.
"""

import numpy as np

N = 512
P = 128
T_STEPS = 20
KS = 11
KP = 258            # padded half-spectrum k-dim (even for fp32r)
NCORES = 8
BATCH = 64
CHANNELS = 3
SPC = BATCH // NCORES  # samples per core
IMGS = SPC * CHANNELS  # images per core

USE_F32R = True

_PROGRAM = None
TRACE = False
LAST_EXEC_NS = None
LAST_TRACE = None


def _build_program():
    import concourse.mybir as mybir
    import concourse.tile as tile
    from concourse import bacc

    f32 = mybir.dt.float32
    f32r = mybir.dt.float32r
    mmdt = f32r if USE_F32R else f32

    nc = bacc.Bacc(
        "TRN2", target_bir_lowering=False, debug=False, num_devices=NCORES
    )
    x_d = nc.dram_tensor("x", [IMGS, N, N], mmdt, kind="ExternalInput").ap()
    fkr_d = nc.dram_tensor("fkr", [SPC, N, KP], f32, kind="ExternalInput").ap()
    fki_d = nc.dram_tensor("fki", [SPC, N, KP], f32, kind="ExternalInput").ap()
    # DFT matrices (all symmetric 512x512): C, S, -S, C+S
    mat_names = ["cmat", "smat", "snmat", "cpsmat"]
    mat_d = {
        nm: nc.dram_tensor(nm, [N, N], mmdt, kind="ExternalInput").ap()
        for nm in mat_names
    }
    out_d = nc.dram_tensor("out", [IMGS, N, N], f32, kind="ExternalOutput").ap()

    with tile.TileContext(nc) as tc:
        with (
            tc.tile_pool(name="mats", bufs=1) as mats,
            tc.tile_pool(name="xsp", bufs=2) as xsp,
            tc.tile_pool(name="outp", bufs=2) as outp,
            tc.tile_pool(name="fkp", bufs=2) as fkp,
            tc.tile_pool(name="apool", bufs=2) as apool,
            tc.tile_pool(name="hpool", bufs=2) as hpool,
            tc.tile_pool(name="zpool", bufs=2) as zpool,
            tc.tile_pool(name="vpool", bufs=2) as vpool,
            tc.tile_pool(name="pw", bufs=2) as pw,
            tc.tile_pool(name="psum", bufs=8, space="PSUM") as psum,
        ):
            # resident DFT matrices, [p, tile, n] layout
            M = {}
            for nm in mat_names:
                mt = mats.tile([P, 4, N], mmdt, name=nm + "_s")
                nc.sync.dma_start(mt[:], mat_d[nm].rearrange("(i p) n -> p i n", p=P))
                M[nm] = mt
            Cs, Ss, Sns, CpSs = M["cmat"], M["smat"], M["snmat"], M["cpsmat"]
            # C/-S rows 130..257, partition-aligned (stage-4 k tail)
            Ck2s = mats.tile([P, N], mmdt, name="ck2_s")
            Snk2s = mats.tile([P, N], mmdt, name="snk2_s")
            nc.sync.dma_start(Ck2s[:], mat_d["cmat"][130:258, :])
            nc.sync.dma_start(Snk2s[:], mat_d["snmat"][130:258, :])

            for s in range(SPC):
                # per-sample spectrum, transposed [l, k] layout, k cols 0..257
                fktr = fkp.tile([P, 4, KP], f32, tag="fktr")
                fkti = fkp.tile([P, 4, KP], f32, tag="fkti")
                nc.sync.dma_start(
                    fktr[:], fkr_d[s].rearrange("(i p) n -> p i n", p=P)
                )
                nc.sync.dma_start(
                    fkti[:], fki_d[s].rearrange("(i p) n -> p i n", p=P)
                )

                for ch in range(CHANNELS):
                    img = s * CHANNELS + ch
                    xs = xsp.tile([P, 4, N], mmdt, tag="xs")
                    nc.sync.dma_start(
                        xs[:], x_d[img].rearrange("(i p) n -> p i n", p=P)
                    )

                    # ---- Stage 1 (data-stationary): A = x^T F, k cols 0..257
                    # A_r = x^T C[:, :258] ; A_i = x^T (-S)[:, :258]
                    Ar = apool.tile([P, 4, KP], mmdt, tag="Ar")
                    Ai = apool.tile([P, 4, KP], mmdt, tag="Ai")
                    for m in range(4):
                        msl = slice(m * P, (m + 1) * P)
                        pa = psum.tile([P, N], f32, tag="ps", name="pa")[:, :KP]
                        pb = psum.tile([P, N], f32, tag="ps", name="pb")[:, :KP]
                        for kk in range(4):
                            nc.tensor.matmul(
                                pa[:], xs[:, kk, msl], Cs[:, kk, 0:KP],
                                start=(kk == 0), stop=(kk == 3),
                            )
                            nc.tensor.matmul(
                                pb[:], xs[:, kk, msl], Sns[:, kk, 0:KP],
                                start=(kk == 0), stop=(kk == 3),
                            )
                        nc.scalar.copy(out=Ar[:, m, :], in_=pa[:])
                        nc.scalar.copy(out=Ai[:, m, :], in_=pb[:])

                    # ---- Stage 2 (matrix-stationary): Ht[l,k] = (F A)[l,k]
                    # Htr = C.Ar + S.Ai ; Hti = C.Ai + (-S).Ar
                    # staged to SBUF (ACT) for the wide pointwise below
                    Hrs = hpool.tile([P, 4, KP], f32, tag="Hrs")
                    His = hpool.tile([P, 4, KP], f32, tag="His")
                    for lm in range(4):
                        lsl = slice(lm * P, (lm + 1) * P)
                        hr = psum.tile([P, N], f32, tag="ps", name="hr")[:, :KP]
                        hi = psum.tile([P, N], f32, tag="ps", name="hi")[:, :KP]
                        for kk in range(4):
                            nc.tensor.matmul(
                                hr[:], Cs[:, kk, lsl], Ar[:, kk, :],
                                start=(kk == 0), stop=False,
                            )
                            nc.tensor.matmul(
                                hi[:], Cs[:, kk, lsl], Ai[:, kk, :],
                                start=(kk == 0), stop=False,
                            )
                        for kk in range(4):
                            nc.tensor.matmul(
                                hr[:], Ss[:, kk, lsl], Ai[:, kk, :],
                                start=False, stop=(kk == 3),
                            )
                            nc.tensor.matmul(
                                hi[:], Sns[:, kk, lsl], Ar[:, kk, :],
                                start=False, stop=(kk == 3),
                            )
                        nc.scalar.copy(out=Hrs[:, lm, :], in_=hr[:])
                        nc.scalar.copy(out=His[:, lm, :], in_=hi[:])

                    # ---- pointwise (wide [128, 4*258] ops):
                    # Ztr = Hr o fr - Hi o fi ; Zti = Hr o fi + Hi o fr
                    # Zq = Zti - Ztr ; Zp = Ztr + Zti   (stage-3 Gauss operands)
                    Ztr = zpool.tile([P, 4, KP], mmdt, tag="Ztr")
                    Zq = zpool.tile([P, 4, KP], mmdt, tag="Zq")
                    Zp = zpool.tile([P, 4, KP], mmdt, tag="Zp")
                    zti = pw.tile([P, 4, KP], f32, tag="zti")
                    tt = pw.tile([P, 4, KP], f32, tag="tt")
                    tu = pw.tile([P, 4, KP], f32, tag="tu")
                    nc.vector.tensor_mul(out=Ztr[:], in0=Hrs[:], in1=fktr[:])
                    nc.gpsimd.tensor_tensor(
                        tt[:], His[:], fkti[:], mybir.AluOpType.mult
                    )
                    nc.vector.tensor_sub(out=Ztr[:], in0=Ztr[:], in1=tt[:])
                    nc.gpsimd.tensor_tensor(
                        zti[:], Hrs[:], fkti[:], mybir.AluOpType.mult
                    )
                    nc.vector.tensor_mul(out=tu[:], in0=His[:], in1=fktr[:])
                    nc.vector.tensor_add(out=zti[:], in0=zti[:], in1=tu[:])
                    nc.vector.tensor_sub(out=Zq[:], in0=zti[:], in1=Ztr[:])
                    nc.vector.tensor_add(out=Zp[:], in0=Ztr[:], in1=zti[:])

                    # ---- Stage 3 (data-stationary, Gauss): V[k,x]
                    # n1 = Ztr^T (C+S) ; n2 = Zq^T C ; n3 = Zp^T S
                    # V_r = n1 - n3 ; V_i = n1 + n2
                    # k M-tiles: 0:128, 128:256, 130:258
                    Vr = vpool.tile([P, 3, N], mmdt, tag="Vr")
                    Vi = vpool.tile([P, 3, N], mmdt, tag="Vi")
                    for km in range(3):
                        koff = (0, 128, 130)[km]
                        ksl = slice(koff, koff + P)
                        n1 = psum.tile([P, N], f32, tag="ps", name="n1")
                        n2 = psum.tile([P, N], f32, tag="ps", name="n2")
                        n3 = psum.tile([P, N], f32, tag="ps", name="n3")
                        for lt in range(4):
                            nc.tensor.matmul(
                                n1[:], Ztr[:, lt, ksl], CpSs[:, lt, :],
                                start=(lt == 0), stop=(lt == 3),
                            )
                        for lt in range(4):
                            nc.tensor.matmul(
                                n2[:], Zq[:, lt, ksl], Cs[:, lt, :],
                                start=(lt == 0), stop=(lt == 3),
                            )
                        for lt in range(4):
                            nc.tensor.matmul(
                                n3[:], Zp[:, lt, ksl], Ss[:, lt, :],
                                start=(lt == 0), stop=(lt == 3),
                            )
                        n1s = pw.tile([P, N], f32, tag="n1s")
                        nc.scalar.copy(out=n1s[:], in_=n1[:])
                        nc.vector.tensor_sub(
                            out=Vr[:, km, :], in0=n1s[:], in1=n3[:]
                        )
                        nc.vector.tensor_add(
                            out=Vi[:, km, :], in0=n1s[:], in1=n2[:]
                        )

                    # ---- Stage 4 (matrix-stationary): natural orientation
                    # out[y,x] = sum_k C[k,y] Vr[k,x] + (-S)[k,y] Vi[k,x]
                    # k tiles: t0 = 0..127, t1 rows 0..1 = 128..129,
                    #          t2 = 130..257 (Ck2s/Snk2s)
                    outs = outp.tile([P, 4, N], f32, tag="outs")
                    for ym in range(4):
                        ysl = slice(ym * P, (ym + 1) * P)
                        po = psum.tile([P, N], f32, tag="ps", name="po")
                        nc.tensor.matmul(
                            po[:], Cs[:, 0, ysl], Vr[:, 0, :],
                            start=True, stop=False,
                        )
                        nc.tensor.matmul(
                            po[:], Cs[0:2, 1, ysl], Vr[0:2, 1, :],
                            start=False, stop=False,
                        )
                        nc.tensor.matmul(
                            po[:], Ck2s[:, ysl], Vr[:, 2, :],
                            start=False, stop=False,
                        )
                        nc.tensor.matmul(
                            po[:], Sns[:, 0, ysl], Vi[:, 0, :],
                            start=False, stop=False,
                        )
                        nc.tensor.matmul(
                            po[:], Sns[0:2, 1, ysl], Vi[0:2, 1, :],
                            start=False, stop=False,
                        )
                        nc.tensor.matmul(
                            po[:], Snk2s[:, ysl], Vi[:, 2, :],
                            start=False, stop=True,
                        )
                        nc.scalar.copy(out=outs[:, ym, :], in_=po[:])
                    nc.sync.dma_start(
                        out_d[img].rearrange("(i p) n -> p i n", p=P), outs[:]
                    )

    nc.compile()
    return nc


def _host_spectra(kernels):
    """Compose step kernels into 21 cumulative half-spectra, transposed to
    [l, k] layout with Hermitian weights and 1/N^2 folded in.
    Returns (FKtr, FKti) f32 [21, 512, KP]."""
    kernels = np.asarray(kernels, dtype=np.float64)
    h = np.zeros((T_STEPS, N, N), np.float64)
    idx = (KS // 2 - np.arange(KS)) % N
    h[:, idx[:, None], idx[None, :]] = kernels
    s_step = np.fft.fft2(h)
    cum = np.ones((T_STEPS + 1, N, N), np.complex128)
    for i in range(1, T_STEPS + 1):
        cum[i] = cum[i - 1] * s_step[i - 1]
    w = np.zeros(KP)
    w[: N // 2 + 1] = 2.0
    w[0] = w[N // 2] = 1.0
    fkt = (cum[:, :KP, :] * w[None, :, None] / float(N * N)).transpose(0, 2, 1)
    return (
        np.ascontiguousarray(fkt.real.astype(np.float32)),
        np.ascontiguousarray(fkt.imag.astype(np.float32)),
    )


def _dft_mats():
    j = np.arange(N)
    ang = 2.0 * np.pi * (np.outer(j, j) % N) / N
    cm = np.cos(ang).astype(np.float32)
    sm = np.sin(ang).astype(np.float32)
    return {
        "cmat": cm,
        "smat": sm,
        "snmat": np.ascontiguousarray(-sm),
        "cpsmat": np.ascontiguousarray(cm + sm),
    }


def kernel(x0, t, kernels):
    global _PROGRAM, LAST_EXEC_NS, LAST_TRACE
    from concourse import bass_utils

    x0 = np.ascontiguousarray(np.asarray(x0), dtype=np.float32)
    tt = np.asarray(t).astype(np.int64)
    fktr_all, fkti_all = _host_spectra(kernels)
    mats = _dft_mats()

    if _PROGRAM is None:
        _PROGRAM = _build_program()
    nc = _PROGRAM

    in_maps = []
    for c in range(NCORES):
        sl = slice(c * SPC, (c + 1) * SPC)
        ts = tt[sl]
        im = {
            "x": np.ascontiguousarray(x0[sl].reshape(IMGS, N, N)),
            "fkr": np.ascontiguousarray(fktr_all[ts]),
            "fki": np.ascontiguousarray(fkti_all[ts]),
        }
        im.update(mats)
        in_maps.append(im)

    res = bass_utils.run_bass_kernel_spmd(
        nc, in_maps, core_ids=list(range(NCORES)), trace=TRACE
    )
    LAST_EXEC_NS = res.exec_time_ns
    if res.instructions_and_trace is not None:
        LAST_TRACE = res.instructions_and_trace[1]
    out = np.empty((BATCH, CHANNELS, N, N), np.float32)
    for c in range(NCORES):
        out[c * SPC : (c + 1) * SPC] = res.results[c]["out"].reshape(
            SPC, CHANNELS, N, N
        )
    return out


# revision 13
# speedup vs baseline: 1.2947x; 1.2947x over previous
"""BlurDegradation kernel for 8x TRN2 NeuronCores.

Math: t[b] successive 11x11 depthwise *circular* convolutions compose into a
single circular convolution whose spectrum is the product of the per-step
spectra. The host composes the (tiny) 20 step-kernels into 21 cumulative
spectra with numpy FFTs and selects per-sample spectrum FK[t[b]]; each device
then computes, per image,  out = Re( F* . (FK o (F x F)) . F* ) / N^2  as four
chained dense matmul stages on the PE array (plus DVE/ACT/GpSimd pointwise
work). Contractions always run over the partition dim, so each stage
implicitly transposes and no PE/DMA transposes are needed.

Optimizations vs the naive 4-stage dense chain:
 - Hermitian half-spectrum: y-frequencies k=0..257 only (258 = even, required
   by fp32r); fold weights (1,2,..,2,1,0) and 1/N^2 live in FK (host-side).
 - Stage 2 is matrix-stationary (resident DFT-matrix weights prefetch cleanly)
   with the k-half axis as the cheap *moving* dim (N=258).
 - Stage 3 is data-stationary with *weight-paired* emission: each fresh
   Z-slice weight load is reused by two consecutive matmuls (the reload is
   skipped), halving the fresh-weight-load penalty.
 - Stage 4 is matrix-stationary with natural [y,x] output; the k-half
   contraction uses three full 128-row tiles (0:128, 128:256, 130:258) with
   the double-counted k=130..255 range pre-halved in FK on the host.
 - H / n1 PSUM tiles are staged to SBUF by the Scalar engine (fast PSUM-bank
   release); the pointwise complex multiply is split across Vector + GpSimd.

Sharding: pure data parallel, 8 samples per core, no cross-core comms.
"""

import numpy as np

N = 512
P = 128
T_STEPS = 20
KS = 11
KP = 258            # padded half-spectrum k-dim (even for fp32r)
NCORES = 8
BATCH = 64
CHANNELS = 3
SPC = BATCH // NCORES  # samples per core
IMGS = SPC * CHANNELS  # images per core

USE_F32R = True

_PROGRAM = None
TRACE = False
LAST_EXEC_NS = None
LAST_TRACE = None


def _build_program():
    import concourse.mybir as mybir
    import concourse.tile as tile
    from concourse import bacc

    f32 = mybir.dt.float32
    f32r = mybir.dt.float32r
    mmdt = f32r if USE_F32R else f32

    nc = bacc.Bacc(
        "TRN2", target_bir_lowering=False, debug=False, num_devices=NCORES
    )
    x_d = nc.dram_tensor("x", [IMGS, N, N], mmdt, kind="ExternalInput").ap()
    fkr_d = nc.dram_tensor("fkr", [SPC, N, KP], f32, kind="ExternalInput").ap()
    fki_d = nc.dram_tensor("fki", [SPC, N, KP], f32, kind="ExternalInput").ap()
    mat_names = ["cmat", "smat", "snmat"]
    mat_d = {
        nm: nc.dram_tensor(nm, [N, N], mmdt, kind="ExternalInput").ap()
        for nm in mat_names
    }
    out_d = nc.dram_tensor("out", [IMGS, N, N], f32, kind="ExternalOutput").ap()

    with tile.TileContext(nc) as tc:
        with (
            tc.tile_pool(name="mats", bufs=1) as mats,
            tc.tile_pool(name="xsp", bufs=2) as xsp,
            tc.tile_pool(name="outp", bufs=2) as outp,
            tc.tile_pool(name="fkp", bufs=2) as fkp,
            tc.tile_pool(name="apool", bufs=2) as apool,
            tc.tile_pool(name="hpool", bufs=2) as hpool,
            tc.tile_pool(name="zpool", bufs=2) as zpool,
            tc.tile_pool(name="vpool", bufs=2) as vpool,
            tc.tile_pool(name="pw", bufs=3) as pw,
            tc.tile_pool(name="psum", bufs=8, space="PSUM") as psum,
        ):
            # resident DFT matrices, [p, tile, n] layout
            M = {}
            for nm in mat_names:
                mt = mats.tile([P, 4, N], mmdt, name=nm + "_s")
                nc.sync.dma_start(mt[:], mat_d[nm].rearrange("(i p) n -> p i n", p=P))
                M[nm] = mt
            Cs, Ss, Sns = M["cmat"], M["smat"], M["snmat"]
            # C/-S rows 130..257, partition-aligned (stage-4 k tail tile)
            Ck2s = mats.tile([P, N], mmdt, name="ck2_s")
            Snk2s = mats.tile([P, N], mmdt, name="snk2_s")
            nc.sync.dma_start(Ck2s[:], mat_d["cmat"][130:258, :])
            nc.sync.dma_start(Snk2s[:], mat_d["snmat"][130:258, :])

            for s in range(SPC):
                # per-sample spectrum, transposed [l, k] layout, k cols 0..257
                fktr = fkp.tile([P, 4, KP], f32, tag="fktr")
                fkti = fkp.tile([P, 4, KP], f32, tag="fkti")
                nc.sync.dma_start(
                    fktr[:], fkr_d[s].rearrange("(i p) n -> p i n", p=P)
                )
                nc.sync.dma_start(
                    fkti[:], fki_d[s].rearrange("(i p) n -> p i n", p=P)
                )

                for ch in range(CHANNELS):
                    img = s * CHANNELS + ch
                    xs = xsp.tile([P, 4, N], mmdt, tag="xs")
                    nc.sync.dma_start(
                        xs[:], x_d[img].rearrange("(i p) n -> p i n", p=P)
                    )

                    # ---- Stage 1 (data-stationary, weight-paired):
                    # A_r = x^T C[:, :258] ; A_i = x^T (-S)[:, :258]
                    Ar = apool.tile([P, 4, KP], mmdt, tag="Ar")
                    Ai = apool.tile([P, 4, KP], mmdt, tag="Ai")
                    for m in range(4):
                        msl = slice(m * P, (m + 1) * P)
                        pa = psum.tile([P, N], f32, tag="ps", name="pa")[:, :KP]
                        pb = psum.tile([P, N], f32, tag="ps", name="pb")[:, :KP]
                        for kk in range(4):
                            nc.tensor.matmul(
                                pa[:], xs[:, kk, msl], Cs[:, kk, 0:KP],
                                start=(kk == 0), stop=(kk == 3),
                            )
                            nc.tensor.matmul(
                                pb[:], xs[:, kk, msl], Sns[:, kk, 0:KP],
                                start=(kk == 0), stop=(kk == 3),
                            )
                        nc.scalar.copy(out=Ar[:, m, :], in_=pa[:])
                        nc.scalar.copy(out=Ai[:, m, :], in_=pb[:])

                    # ---- Stage 2 (matrix-stationary): Ht[l,k] = (F A)[l,k]
                    # Htr = C.Ar + S.Ai ; Hti = C.Ai + (-S).Ar
                    # staged to SBUF (ACT); pointwise per l-tile below
                    Ztr = zpool.tile([P, 4, KP], mmdt, tag="Ztr")
                    Zti = zpool.tile([P, 4, KP], mmdt, tag="Zti")
                    for lm in range(4):
                        lsl = slice(lm * P, (lm + 1) * P)
                        hr = psum.tile([P, N], f32, tag="ps", name="hr")[:, :KP]
                        hi = psum.tile([P, N], f32, tag="ps", name="hi")[:, :KP]
                        for kk in range(4):
                            nc.tensor.matmul(
                                hr[:], Cs[:, kk, lsl], Ar[:, kk, :],
                                start=(kk == 0), stop=False,
                            )
                            nc.tensor.matmul(
                                hi[:], Cs[:, kk, lsl], Ai[:, kk, :],
                                start=(kk == 0), stop=False,
                            )
                        for kk in range(4):
                            nc.tensor.matmul(
                                hr[:], Ss[:, kk, lsl], Ai[:, kk, :],
                                start=False, stop=(kk == 3),
                            )
                            nc.tensor.matmul(
                                hi[:], Sns[:, kk, lsl], Ar[:, kk, :],
                                start=False, stop=(kk == 3),
                            )
                        hrs = pw.tile([P, KP], f32, tag="hrs")
                        his = pw.tile([P, KP], f32, tag="his")
                        nc.scalar.copy(out=hrs[:], in_=hr[:])
                        nc.scalar.copy(out=his[:], in_=hi[:])
                        # pointwise: Ztr = hr o fr - hi o fi
                        #            Zti = hr o fi + hi o fr
                        fr = fktr[:, lm, :]
                        fi = fkti[:, lm, :]
                        tt = pw.tile([P, KP], f32, tag="tt")
                        tu = pw.tile([P, KP], f32, tag="tu")
                        ztr = Ztr[:, lm, :]
                        zti = Zti[:, lm, :]
                        nc.vector.tensor_mul(out=ztr, in0=hrs[:], in1=fr)
                        nc.gpsimd.tensor_tensor(
                            tt[:], his[:], fi, mybir.AluOpType.mult
                        )
                        nc.vector.tensor_sub(out=ztr, in0=ztr, in1=tt[:])
                        nc.gpsimd.tensor_tensor(
                            tu[:], hrs[:], fi, mybir.AluOpType.mult
                        )
                        nc.vector.tensor_mul(out=zti, in0=his[:], in1=fr)
                        nc.vector.tensor_add(out=zti, in0=zti, in1=tu[:])

                    # ---- Stage 3 (data-stationary, weight-paired direct):
                    # V_r = Ztr^T C + Zti^T (-S) ; V_i = Ztr^T S + Zti^T C
                    # k M-tiles: 0:128, 128:256, 130:258
                    Vr = vpool.tile([P, 3, N], mmdt, tag="Vr")
                    Vi = vpool.tile([P, 3, N], mmdt, tag="Vi")
                    for km in range(3):
                        koff = (0, 128, 130)[km]
                        ksl = slice(koff, koff + P)
                        nvr = psum.tile([P, N], f32, tag="ps", name="nvr")
                        nvi = psum.tile([P, N], f32, tag="ps", name="nvi")
                        for lt in range(4):
                            nc.tensor.matmul(
                                nvr[:], Ztr[:, lt, ksl], Cs[:, lt, :],
                                start=(lt == 0), stop=False,
                            )
                            nc.tensor.matmul(
                                nvi[:], Ztr[:, lt, ksl], Ss[:, lt, :],
                                start=(lt == 0), stop=False,
                            )
                        for lt in range(4):
                            nc.tensor.matmul(
                                nvr[:], Zti[:, lt, ksl], Sns[:, lt, :],
                                start=False, stop=(lt == 3),
                            )
                            nc.tensor.matmul(
                                nvi[:], Zti[:, lt, ksl], Cs[:, lt, :],
                                start=False, stop=(lt == 3),
                            )
                        nc.any.tensor_copy(out=Vr[:, km, :], in_=nvr[:])
                        nc.any.tensor_copy(out=Vi[:, km, :], in_=nvi[:])

                    # ---- Stage 4 (matrix-stationary, natural orientation):
                    # out[y,x] = sum_k C[k,y] Vr[k,x] + (-S)[k,y] Vi[k,x]
                    # k tiles: 0:128, 128:256, 130:258 (FK pre-halved on the
                    # double-counted 130..255 range)
                    outs = outp.tile([P, 4, N], f32, tag="outs")
                    for ym in range(4):
                        ysl = slice(ym * P, (ym + 1) * P)
                        po = psum.tile([P, N], f32, tag="ps", name="po")
                        nc.tensor.matmul(
                            po[:], Cs[:, 0, ysl], Vr[:, 0, :],
                            start=True, stop=False,
                        )
                        nc.tensor.matmul(
                            po[:], Cs[:, 1, ysl], Vr[:, 1, :],
                            start=False, stop=False,
                        )
                        nc.tensor.matmul(
                            po[:], Ck2s[:, ysl], Vr[:, 2, :],
                            start=False, stop=False,
                        )
                        nc.tensor.matmul(
                            po[:], Sns[:, 0, ysl], Vi[:, 0, :],
                            start=False, stop=False,
                        )
                        nc.tensor.matmul(
                            po[:], Sns[:, 1, ysl], Vi[:, 1, :],
                            start=False, stop=False,
                        )
                        nc.tensor.matmul(
                            po[:], Snk2s[:, ysl], Vi[:, 2, :],
                            start=False, stop=True,
                        )
                        nc.any.tensor_copy(out=outs[:, ym, :], in_=po[:])
                    nc.sync.dma_start(
                        out_d[img].rearrange("(i p) n -> p i n", p=P), outs[:]
                    )

    nc.compile()
    return nc


def _host_spectra(kernels):
    """Compose step kernels into 21 cumulative half-spectra, transposed to
    [l, k] layout with Hermitian weights, 1/N^2, and the stage-4
    double-count halving folded in. Returns (FKtr, FKti) f32 [21, 512, KP]."""
    kernels = np.asarray(kernels, dtype=np.float64)
    h = np.zeros((T_STEPS, N, N), np.float64)
    idx = (KS // 2 - np.arange(KS)) % N
    h[:, idx[:, None], idx[None, :]] = kernels
    s_step = np.fft.fft2(h)
    cum = np.ones((T_STEPS + 1, N, N), np.complex128)
    for i in range(1, T_STEPS + 1):
        cum[i] = cum[i - 1] * s_step[i - 1]
    w = np.zeros(KP)
    w[: N // 2 + 1] = 2.0
    w[0] = w[N // 2] = 1.0
    fkt = (cum[:, :KP, :] * w[None, :, None] / float(N * N)).transpose(0, 2, 1)
    half = np.ones(KP)
    half[130:256] = 0.5  # k rows 130..255 appear in both stage-4 k-tiles
    fkt = fkt * half[None, None, :]
    return (
        np.ascontiguousarray(fkt.real.astype(np.float32)),
        np.ascontiguousarray(fkt.imag.astype(np.float32)),
    )


def _dft_mats():
    j = np.arange(N)
    ang = 2.0 * np.pi * (np.outer(j, j) % N) / N
    cm = np.cos(ang).astype(np.float32)
    sm = np.sin(ang).astype(np.float32)
    return {
        "cmat": cm,
        "smat": sm,
        "snmat": np.ascontiguousarray(-sm),
    }


def kernel(x0, t, kernels):
    global _PROGRAM, LAST_EXEC_NS, LAST_TRACE
    from concourse import bass_utils

    x0 = np.ascontiguousarray(np.asarray(x0), dtype=np.float32)
    tt = np.asarray(t).astype(np.int64)
    fktr_all, fkti_all = _host_spectra(kernels)
    mats = _dft_mats()

    if _PROGRAM is None:
        _PROGRAM = _build_program()
    nc = _PROGRAM

    in_maps = []
    for c in range(NCORES):
        sl = slice(c * SPC, (c + 1) * SPC)
        ts = tt[sl]
        im = {
            "x": np.ascontiguousarray(x0[sl].reshape(IMGS, N, N)),
            "fkr": np.ascontiguousarray(fktr_all[ts]),
            "fki": np.ascontiguousarray(fkti_all[ts]),
        }
        im.update(mats)
        in_maps.append(im)

    res = bass_utils.run_bass_kernel_spmd(
        nc, in_maps, core_ids=list(range(NCORES)), trace=TRACE
    )
    LAST_EXEC_NS = res.exec_time_ns
    if res.instructions_and_trace is not None:
        LAST_TRACE = res.instructions_and_trace[1]
    out = np.empty((BATCH, CHANNELS, N, N), np.float32)
    for c in range(NCORES):
        out[c * SPC : (c + 1) * SPC] = res.results[c]["out"].reshape(
            SPC, CHANNELS, N, N
        )
    return out


# revision 14
# speedup vs baseline: 1.4161x; 1.0938x over previous
"""BlurDegradation kernel for 8x TRN2 NeuronCores.

Math: t[b] successive 11x11 depthwise *circular* convolutions compose into a
single circular convolution whose spectrum is the product of the per-step
spectra. The host composes the (tiny) 20 step-kernels into 21 cumulative
spectra with numpy FFTs and selects per-sample spectrum FK[t[b]]; each device
then computes, per image,  out = Re( F* . (FK o (F x F)) . F* ) / N^2  as four
chained dense matmul stages on the PE array (plus DVE/ACT/GpSimd pointwise
work). Contractions always run over the partition dim, so each stage
implicitly transposes and no PE/DMA transposes are needed.

Optimizations vs the naive 4-stage dense chain:
 - Hermitian half-spectrum: y-frequencies k=0..257 only (258 = even, required
   by fp32r); fold weights (1,2,..,2,1,0) and 1/N^2 live in FK (host-side).
 - Stage 2 is matrix-stationary (resident DFT-matrix weights prefetch cleanly)
   with the k-half axis as the cheap *moving* dim (N=258).
 - Stage 3 is data-stationary with *weight-paired* emission: each fresh
   Z-slice weight load is reused by two consecutive matmuls (the reload is
   skipped), halving the fresh-weight-load penalty.
 - Stage 4 is matrix-stationary with natural [y,x] output; the k-half
   contraction uses three full 128-row tiles (0:128, 128:256, 130:258) with
   the double-counted k=130..255 range pre-halved in FK on the host.
 - H / n1 PSUM tiles are staged to SBUF by the Scalar engine (fast PSUM-bank
   release); the pointwise complex multiply is split across Vector + GpSimd.

Sharding: pure data parallel, 8 samples per core, no cross-core comms.
"""

import numpy as np

N = 512
P = 128
T_STEPS = 20
KS = 11
KP = 258            # padded half-spectrum k-dim (even for fp32r)
NCORES = 8
BATCH = 64
CHANNELS = 3
SPC = BATCH // NCORES  # samples per core
IMGS = SPC * CHANNELS  # images per core

USE_F32R = True

_PROGRAM = None
TRACE = False
LAST_EXEC_NS = None
LAST_TRACE = None


def _build_program():
    import concourse.mybir as mybir
    import concourse.tile as tile
    from concourse import bacc

    f32 = mybir.dt.float32
    f32r = mybir.dt.float32r
    mmdt = f32r if USE_F32R else f32

    nc = bacc.Bacc(
        "TRN2", target_bir_lowering=False, debug=False, num_devices=NCORES
    )
    x_d = nc.dram_tensor("x", [IMGS, N, N], mmdt, kind="ExternalInput").ap()
    fkr_d = nc.dram_tensor("fkr", [SPC, N, KP], f32, kind="ExternalInput").ap()
    fki_d = nc.dram_tensor("fki", [SPC, N, KP], f32, kind="ExternalInput").ap()
    mat_names = ["cmat", "smat", "snmat"]
    mat_d = {
        nm: nc.dram_tensor(nm, [N, N], mmdt, kind="ExternalInput").ap()
        for nm in mat_names
    }
    out_d = nc.dram_tensor("out", [IMGS, N, N], f32, kind="ExternalOutput").ap()

    with tile.TileContext(nc) as tc:
        with (
            tc.tile_pool(name="mats", bufs=1) as mats,
            tc.tile_pool(name="xsp", bufs=2) as xsp,
            tc.tile_pool(name="outp", bufs=2) as outp,
            tc.tile_pool(name="fkp", bufs=2) as fkp,
            tc.tile_pool(name="apool", bufs=2) as apool,
            tc.tile_pool(name="hpool", bufs=2) as hpool,
            tc.tile_pool(name="zpool", bufs=2) as zpool,
            tc.tile_pool(name="vpool", bufs=2) as vpool,
            tc.tile_pool(name="pw", bufs=3) as pw,
            tc.tile_pool(name="psum", bufs=8, space="PSUM") as psum,
        ):
            # resident DFT matrices, [p, tile, n] layout
            M = {}
            for nm in mat_names:
                mt = mats.tile([P, 4, N], mmdt, name=nm + "_s")
                nc.sync.dma_start(mt[:], mat_d[nm].rearrange("(i p) n -> p i n", p=P))
                M[nm] = mt
            Cs, Ss, Sns = M["cmat"], M["smat"], M["snmat"]
            # C/-S rows 130..257, partition-aligned (stage-4 k tail tile)
            Ck2s = mats.tile([P, N], mmdt, name="ck2_s")
            Snk2s = mats.tile([P, N], mmdt, name="snk2_s")
            nc.sync.dma_start(Ck2s[:], mat_d["cmat"][130:258, :])
            nc.sync.dma_start(Snk2s[:], mat_d["snmat"][130:258, :])

            def emit_st4(Vr, Vi, img):
                # ---- Stage 4 (matrix-stationary, natural orientation):
                # out[y,x] = sum_k C[k,y] Vr[k,x] + (-S)[k,y] Vi[k,x]
                # k tiles: 0:128, 128:256, 130:258 (FK pre-halved on the
                # double-counted 130..255 range)
                outs = outp.tile([P, 4, N], f32, tag="outs")
                for ym in range(4):
                    ysl = slice(ym * P, (ym + 1) * P)
                    po = psum.tile([P, N], f32, tag="ps", name="po")
                    nc.tensor.matmul(
                        po[:], Cs[:, 0, ysl], Vr[:, 0, :],
                        start=True, stop=False,
                    )
                    nc.tensor.matmul(
                        po[:], Cs[:, 1, ysl], Vr[:, 1, :],
                        start=False, stop=False,
                    )
                    nc.tensor.matmul(
                        po[:], Ck2s[:, ysl], Vr[:, 2, :],
                        start=False, stop=False,
                    )
                    nc.tensor.matmul(
                        po[:], Sns[:, 0, ysl], Vi[:, 0, :],
                        start=False, stop=False,
                    )
                    nc.tensor.matmul(
                        po[:], Sns[:, 1, ysl], Vi[:, 1, :],
                        start=False, stop=False,
                    )
                    nc.tensor.matmul(
                        po[:], Snk2s[:, ysl], Vi[:, 2, :],
                        start=False, stop=True,
                    )
                    nc.any.tensor_copy(out=outs[:, ym, :], in_=po[:])
                nc.sync.dma_start(
                    out_d[img].rearrange("(i p) n -> p i n", p=P), outs[:]
                )

            pending = None  # (Vr, Vi, img) of the previous image

            for s in range(SPC):
                # per-sample spectrum, transposed [l, k] layout, k cols 0..257
                fktr = fkp.tile([P, 4, KP], f32, tag="fktr")
                fkti = fkp.tile([P, 4, KP], f32, tag="fkti")
                nc.sync.dma_start(
                    fktr[:], fkr_d[s].rearrange("(i p) n -> p i n", p=P)
                )
                nc.sync.dma_start(
                    fkti[:], fki_d[s].rearrange("(i p) n -> p i n", p=P)
                )

                for ch in range(CHANNELS):
                    img = s * CHANNELS + ch
                    xs = xsp.tile([P, 4, N], mmdt, tag="xs")
                    nc.sync.dma_start(
                        xs[:], x_d[img].rearrange("(i p) n -> p i n", p=P)
                    )

                    # ---- Stage 1 (data-stationary, weight-paired):
                    # A_r = x^T C[:, :258] ; A_i = x^T (-S)[:, :258]
                    Ar = apool.tile([P, 4, KP], mmdt, tag="Ar")
                    Ai = apool.tile([P, 4, KP], mmdt, tag="Ai")
                    for m in range(4):
                        msl = slice(m * P, (m + 1) * P)
                        pa = psum.tile([P, N], f32, tag="ps", name="pa")[:, :KP]
                        pb = psum.tile([P, N], f32, tag="ps", name="pb")[:, :KP]
                        for kk in range(4):
                            nc.tensor.matmul(
                                pa[:], xs[:, kk, msl], Cs[:, kk, 0:KP],
                                start=(kk == 0), stop=(kk == 3),
                            )
                            nc.tensor.matmul(
                                pb[:], xs[:, kk, msl], Sns[:, kk, 0:KP],
                                start=(kk == 0), stop=(kk == 3),
                            )
                        nc.scalar.copy(out=Ar[:, m, :], in_=pa[:])
                        nc.scalar.copy(out=Ai[:, m, :], in_=pb[:])

                    # ---- Stage 2 (matrix-stationary): Ht[l,k] = (F A)[l,k]
                    # Htr = C.Ar + S.Ai ; Hti = C.Ai + (-S).Ar
                    # staged to SBUF (ACT); pointwise per l-tile below
                    Ztr = zpool.tile([P, 4, KP], mmdt, tag="Ztr")
                    Zti = zpool.tile([P, 4, KP], mmdt, tag="Zti")
                    for lm in range(4):
                        lsl = slice(lm * P, (lm + 1) * P)
                        hr = psum.tile([P, N], f32, tag="ps", name="hr")[:, :KP]
                        hi = psum.tile([P, N], f32, tag="ps", name="hi")[:, :KP]
                        for kk in range(4):
                            nc.tensor.matmul(
                                hr[:], Cs[:, kk, lsl], Ar[:, kk, :],
                                start=(kk == 0), stop=False,
                            )
                            nc.tensor.matmul(
                                hi[:], Cs[:, kk, lsl], Ai[:, kk, :],
                                start=(kk == 0), stop=False,
                            )
                        for kk in range(4):
                            nc.tensor.matmul(
                                hr[:], Ss[:, kk, lsl], Ai[:, kk, :],
                                start=False, stop=(kk == 3),
                            )
                            nc.tensor.matmul(
                                hi[:], Sns[:, kk, lsl], Ar[:, kk, :],
                                start=False, stop=(kk == 3),
                            )
                        hrs = pw.tile([P, KP], f32, tag="hrs")
                        his = pw.tile([P, KP], f32, tag="his")
                        nc.scalar.copy(out=hrs[:], in_=hr[:])
                        nc.scalar.copy(out=his[:], in_=hi[:])
                        # pointwise: Ztr = hr o fr - hi o fi
                        #            Zti = hr o fi + hi o fr
                        fr = fktr[:, lm, :]
                        fi = fkti[:, lm, :]
                        tt = pw.tile([P, KP], f32, tag="tt")
                        tu = pw.tile([P, KP], f32, tag="tu")
                        ztr = Ztr[:, lm, :]
                        zti = Zti[:, lm, :]
                        nc.vector.tensor_mul(out=ztr, in0=hrs[:], in1=fr)
                        nc.gpsimd.tensor_tensor(
                            tt[:], his[:], fi, mybir.AluOpType.mult
                        )
                        nc.vector.tensor_sub(out=ztr, in0=ztr, in1=tt[:])
                        nc.gpsimd.tensor_tensor(
                            tu[:], hrs[:], fi, mybir.AluOpType.mult
                        )
                        nc.vector.tensor_mul(out=zti, in0=his[:], in1=fr)
                        nc.vector.tensor_add(out=zti, in0=zti, in1=tu[:])

                    # fill the pointwise-latency bubble with the previous
                    # image's stage 4 (independent PE work)
                    if pending is not None:
                        emit_st4(*pending)
                        pending = None

                    # ---- Stage 3 (data-stationary, weight-paired direct):
                    # V_r = Ztr^T C + Zti^T (-S) ; V_i = Ztr^T S + Zti^T C
                    # k M-tiles: 0:128, 128:256, 130:258
                    Vr = vpool.tile([P, 3, N], mmdt, tag="Vr")
                    Vi = vpool.tile([P, 3, N], mmdt, tag="Vi")
                    for km in range(3):
                        koff = (0, 128, 130)[km]
                        ksl = slice(koff, koff + P)
                        nvr = psum.tile([P, N], f32, tag="ps", name="nvr")
                        nvi = psum.tile([P, N], f32, tag="ps", name="nvi")
                        for lt in range(4):
                            nc.tensor.matmul(
                                nvr[:], Ztr[:, lt, ksl], Cs[:, lt, :],
                                start=(lt == 0), stop=False,
                            )
                            nc.tensor.matmul(
                                nvi[:], Ztr[:, lt, ksl], Ss[:, lt, :],
                                start=(lt == 0), stop=False,
                            )
                        for lt in range(4):
                            nc.tensor.matmul(
                                nvr[:], Zti[:, lt, ksl], Sns[:, lt, :],
                                start=False, stop=(lt == 3),
                            )
                            nc.tensor.matmul(
                                nvi[:], Zti[:, lt, ksl], Cs[:, lt, :],
                                start=False, stop=(lt == 3),
                            )
                        nc.any.tensor_copy(out=Vr[:, km, :], in_=nvr[:])
                        nc.any.tensor_copy(out=Vi[:, km, :], in_=nvi[:])

                    pending = (Vr, Vi, img)


            if pending is not None:
                emit_st4(*pending)

    nc.compile()
    return nc


def _host_spectra(kernels):
    """Compose step kernels into 21 cumulative half-spectra, transposed to
    [l, k] layout with Hermitian weights, 1/N^2, and the stage-4
    double-count halving folded in. Returns (FKtr, FKti) f32 [21, 512, KP]."""
    kernels = np.asarray(kernels, dtype=np.float64)
    h = np.zeros((T_STEPS, N, N), np.float64)
    idx = (KS // 2 - np.arange(KS)) % N
    h[:, idx[:, None], idx[None, :]] = kernels
    s_step = np.fft.fft2(h)
    cum = np.ones((T_STEPS + 1, N, N), np.complex128)
    for i in range(1, T_STEPS + 1):
        cum[i] = cum[i - 1] * s_step[i - 1]
    w = np.zeros(KP)
    w[: N // 2 + 1] = 2.0
    w[0] = w[N // 2] = 1.0
    fkt = (cum[:, :KP, :] * w[None, :, None] / float(N * N)).transpose(0, 2, 1)
    half = np.ones(KP)
    half[130:256] = 0.5  # k rows 130..255 appear in both stage-4 k-tiles
    fkt = fkt * half[None, None, :]
    return (
        np.ascontiguousarray(fkt.real.astype(np.float32)),
        np.ascontiguousarray(fkt.imag.astype(np.float32)),
    )


def _dft_mats():
    j = np.arange(N)
    ang = 2.0 * np.pi * (np.outer(j, j) % N) / N
    cm = np.cos(ang).astype(np.float32)
    sm = np.sin(ang).astype(np.float32)
    return {
        "cmat": cm,
        "smat": sm,
        "snmat": np.ascontiguousarray(-sm),
    }


def kernel(x0, t, kernels):
    global _PROGRAM, LAST_EXEC_NS, LAST_TRACE
    from concourse import bass_utils

    x0 = np.ascontiguousarray(np.asarray(x0), dtype=np.float32)
    tt = np.asarray(t).astype(np.int64)
    fktr_all, fkti_all = _host_spectra(kernels)
    mats = _dft_mats()

    if _PROGRAM is None:
        _PROGRAM = _build_program()
    nc = _PROGRAM

    in_maps = []
    for c in range(NCORES):
        sl = slice(c * SPC, (c + 1) * SPC)
        ts = tt[sl]
        im = {
            "x": np.ascontiguousarray(x0[sl].reshape(IMGS, N, N)),
            "fkr": np.ascontiguousarray(fktr_all[ts]),
            "fki": np.ascontiguousarray(fkti_all[ts]),
        }
        im.update(mats)
        in_maps.append(im)

    res = bass_utils.run_bass_kernel_spmd(
        nc, in_maps, core_ids=list(range(NCORES)), trace=TRACE
    )
    LAST_EXEC_NS = res.exec_time_ns
    if res.instructions_and_trace is not None:
        LAST_TRACE = res.instructions_and_trace[1]
    out = np.empty((BATCH, CHANNELS, N, N), np.float32)
    for c in range(NCORES):
        out[c * SPC : (c + 1) * SPC] = res.results[c]["out"].reshape(
            SPC, CHANNELS, N, N
        )
    return out
